# revision 16
# baseline (speedup 1.0000x reference)
"""RBF kernel matrix on 8 TRN2 NeuronCores.

Computes out[i, j] = exp(-gamma * (||x_i||^2 + ||y_j||^2 - 2 x_i.y_j))
with gamma = softplus(MLP(x[0])) + 1e-6, as a Bass/Tile SPMD kernel.

Sharding: rows of x across the 8 cores (1024 rows each); y and the tiny
gamma-net parameters are replicated.  Each core computes its (1024, 8192)
slab of the output; the host concatenates the slabs and widens bf16->f32
(the on-device pipeline is bf16 end-to-end after the exp, so the widening
is exact -- same numbers, half the HBM write traffic).

Per-core device pipeline (keeps the PE free of norm-handling matmuls):
  - gamma chain (TensorE f32 matmuls + ACT relu / exp / ln -> softplus)
    from one packed 19 KB parameter DMA
  - Ey[j] = exp(-gamma*||y_j||^2) built once per panel: DVE squares y^T,
    K=128 matmuls against a constant (-0.5) stationary matrix reduce
    them, ACT exponentiates with the per-partition 2*gamma scale
  - x-row norms from native x via DVE square+accum -> per-partition bias
  - main loop per [128, 2048] tile: psum = x.y (bf16 K=128 matmuls), ACT
    stage = exp(2g*psum - g*||x_i||^2) in bf16, DVE stage *= Ey (bf16 2x
    mode), HWDGE DMA of the bf16 tile to DRAM.
The multiplicative split exp(a+b) = exp(a)*exp(b) is safe here: both
factors are ~exp(-gamma*O(d)) << 1 for this input distribution, so
neither factor can overflow on its own.
"""

import numpy as np
import ml_dtypes

import concourse.bacc as bacc
import concourse.bass as bass  # noqa: F401
import concourse.mybir as mybir
import concourse.tile as tile
from concourse.bass_utils import run_bass_kernel_spmd

N_CORES = 8
N, M, D = 8192, 8192, 256
N_SH = N // N_CORES  # rows of x per core
HID = 16
P = 128
KC = D // P  # k-chunks (2)
MT = N_SH // P  # m-tiles per core (8)
YP = 1024  # y columns per input-DMA piece
NP = M // YP  # pieces (8)
PANEL = 2048  # main-loop columns per panel / psum tile
NPAN = M // PANEL  # panels (4)
GP_COLS = 37  # packed gamma-net params: w1t|w1t|x0|x0|b1|w2t|b2

F32 = mybir.dt.float32
BF16 = mybir.dt.bfloat16
AF = mybir.ActivationFunctionType
ALU = mybir.AluOpType

_NC = None
LAST_RESULT = None


def _ensure_ntff_hook():
    """Register an ``antenv.axon_hooks`` shim if the image lacks it.

    ``run_bass_kernel_spmd(trace=True)`` under axon imports
    ``antenv.axon_hooks.get_axon_ntff_profile_hook``; some images miss the
    module, which would crash tracing.  Recreate the boot-script hook via
    ctypes against libaxon_pjrt.so, degrading to hook=None when absent.
    """
    import contextlib
    import ctypes
    import os
    import sys
    import types

    try:
        import antenv.axon_hooks  # noqa: F401
        return
    except ImportError:
        pass

    hook = None
    so_path = "/opt/axon/libaxon_pjrt.so"
    if os.path.exists(so_path):
        try:
            lib = ctypes.CDLL(so_path)
            if hasattr(lib, "axon_start_nrt_profile"):
                lib.axon_start_nrt_profile.argtypes = [
                    ctypes.POINTER(ctypes.c_int64), ctypes.c_size_t]
                lib.axon_start_nrt_profile.restype = ctypes.c_int64
                lib.axon_stop_nrt_profile.argtypes = [ctypes.c_char_p]
                lib.axon_stop_nrt_profile.restype = ctypes.c_int64

                @contextlib.contextmanager
                def _hook(output_dir, device_ids):
                    import jax
                    jax.devices()
                    if device_ids:
                        ids = (ctypes.c_int64 * len(device_ids))(*device_ids)
                        rc = lib.axon_start_nrt_profile(ids, len(device_ids))
                    else:
                        rc = lib.axon_start_nrt_profile(None, 0)
                    if rc != 0:
                        raise RuntimeError(f"axon_start_nrt_profile rc={rc}")
                    try:
                        yield
                    finally:
                        n = lib.axon_stop_nrt_profile(str(output_dir).encode())
                        if n <= 0:
                            print(f"ntff profile capture wrote {n} files",
                                  file=sys.stderr)

                hook = _hook
        except OSError:
            hook = None

    mod = types.ModuleType("antenv.axon_hooks")
    mod._hook = hook
    mod.get_axon_ntff_profile_hook = lambda: mod._hook

    def _set(h):
        mod._hook = h

    mod.set_axon_ntff_profile_hook = _set
    sys.modules["antenv.axon_hooks"] = mod
    try:
        import antenv
        antenv.axon_hooks = mod
    except ImportError:
        pass


_ensure_ntff_hook()


def _build_nc():
    nc = bacc.Bacc("TRN2", target_bir_lowering=False, debug=False,
                   num_devices=N_CORES)

    gp_d = nc.dram_tensor("gp", [P, GP_COLS], F32, kind="ExternalInput")
    xt_d = nc.dram_tensor("xt", [P, KC, N_SH], BF16, kind="ExternalInput")
    xr_d = nc.dram_tensor("xr", [P, MT, D], BF16, kind="ExternalInput")
    yt_d = nc.dram_tensor("yt", [NP, P, KC, YP], BF16, kind="ExternalInput")
    out_d = nc.dram_tensor("out", [N_SH, M], BF16, kind="ExternalOutput")

    with tile.TileContext(nc) as tc:
        with (
            tc.tile_pool(name="const", bufs=1) as const,
            tc.tile_pool(name="work", bufs=3) as work,
            tc.tile_pool(name="stage", bufs=3) as stage_pool,
            tc.tile_pool(name="ps", bufs=2, space="PSUM") as ps_pool,
        ):
            # ------------- input DMAs (gamma/y on the Sync HWDGE ring, ----
            # ------------- x tensors on the ACT ring) ---------------------
            gp = const.tile([P, GP_COLS], F32)
            nc.sync.dma_start(gp[:], gp_d[:])
            yT_sb = const.tile([P, NP, KC, YP], BF16)
            for c in range(NP):
                nc.sync.dma_start(yT_sb[:, c], yt_d[c])
            xT_sb = const.tile([P, KC, N_SH], BF16)
            nc.scalar.dma_start(xT_sb[:], xt_d[:])
            xr_sb = const.tile([P, MT, D], BF16)
            nc.scalar.dma_start(xr_sb[:], xr_d[:])

            # DVE constants first: no dependencies, run during the boot gap
            ones_row = const.tile([1, P], F32)
            nc.vector.memset(ones_row[:], 1.0)
            negh = const.tile([P, P], BF16)  # stationary -0.5 for ||y||^2
            nc.vector.memset(negh[:], -0.5)
            wrhs = const.tile([P, 512], BF16)  # junk rhs for PE warm-up
            nc.vector.memset(wrhs[:], 0.0)

            # ---------------- gamma chain ----------------
            ps_h = ps_pool.tile([HID, 1], F32, tag="mm")
            for k in range(KC):
                nc.tensor.matmul(ps_h[:], gp[:, k * HID:(k + 1) * HID],
                                 gp[:, 32 + k:33 + k],
                                 start=(k == 0), stop=(k == KC - 1))
            h_sb = const.tile([HID, 1], F32)  # relu(W1 x0 + b1) on the DVE
            nc.vector.tensor_scalar(h_sb[:], ps_h[:], gp[0:HID, 34:35], 0.0,
                                    ALU.add, ALU.max)

            ps_z = ps_pool.tile([1, 1], F32, tag="mm")
            nc.tensor.matmul(ps_z[:], gp[0:HID, 35:36], h_sb[:],
                             start=True, stop=True)
            u_sb = const.tile([1, 1], F32)
            nc.scalar.activation(u_sb[:], ps_z[:], AF.Exp, bias=gp[0:1, 36:37])
            s_sb = const.tile([1, 1], F32)  # softplus(z) = ln(1 + e^z)
            nc.scalar.activation(s_sb[:], u_sb[:], AF.Ln, bias=1.0)
            # dummy exp: forces the exp table-set reload (after Ln evicted
            # it) to happen here, off the Ey critical path
            dummy_e = const.tile([1, 1], F32)
            nc.scalar.activation(dummy_e[:], u_sb[:], AF.Exp)

            ps_g = ps_pool.tile([P, 1], F32, tag="mm")
            nc.tensor.matmul(ps_g[:], ones_row[:], s_sb[:], start=True, stop=True)

            # PE warm-up: ~3.4us of junk matmuls during the otherwise-idle
            # input-load window flips the HAM clock gate to 8/8 before the
            # first real K=128 matmuls (negh is the only dep; no readers)
            warm_ps = ps_pool.tile([P, 512], F32, tag="mm")
            for w in range(8):
                nc.tensor.matmul(warm_ps[:], negh[:], wrhs[:],
                                 start=True, stop=True, skip_group_check=True)

            negg_f = const.tile([P, 1], F32)     # -gamma on every partition
            nc.vector.tensor_scalar(negg_f[:], ps_g[:], -1.0, -1e-6,
                                    ALU.mult, ALU.add)
            p2g_f = const.tile([P, 1], F32)      # +2*gamma
            nc.vector.tensor_scalar(p2g_f[:], ps_g[:], 2.0, 2e-6,
                                    ALU.mult, ALU.add)

            xn = const.tile([P, MT], F32)
            negxn = const.tile([P, MT], F32)     # -gamma * ||x_i||^2
            Eyb = const.tile([P, NPAN, PANEL], BF16)

            # ---------------- panels: Ey prep + main loop -----------------
            for p in range(NPAN):
                # sqy + (-0.5)-matmuls for Ey of this panel's columns; these
                # need neither gamma nor x, so they fill the prolog
                ps_b = ps_pool.tile([P, PANEL], F32, tag="mm")
                for c in range(PANEL // YP):
                    piece = p * (PANEL // YP) + c
                    sqy = work.tile([P, KC, YP], BF16, tag="sqy")
                    nc.vector.tensor_tensor(sqy[:], yT_sb[:, piece],
                                            yT_sb[:, piece], ALU.mult)
                    for k in range(KC):
                        for j in range(YP // 512):
                            nc.tensor.matmul(
                                ps_b[:, c * YP + j * 512:c * YP + (j + 1) * 512],
                                negh[:], sqy[:, k, j * 512:(j + 1) * 512],
                                start=(k == 0), stop=(k == KC - 1))

                # Ey[j] = exp(-gamma*||y_j||^2) for this panel's columns
                nc.scalar.activation(Eyb[:, p], ps_b[:], AF.Exp,
                                     scale=p2g_f[:])

                if p == 0:
                    # x-row norms -> per-partition bias (needed by the first
                    # main-loop exp; emitted after panel-0 sqy so the DVE
                    # queue never stalls on the slower xr DMA)
                    for m in range(MT):
                        sq_scr = work.tile([P, D], F32, tag="sqx")
                        nc.vector.scalar_tensor_tensor(
                            sq_scr[:], xr_sb[:, m], 1.0, xr_sb[:, m],
                            ALU.mult, ALU.mult, accum_out=xn[:, m:m + 1])
                    nc.vector.tensor_scalar(negxn[:], xn[:], negg_f[:], None,
                                            ALU.mult)

                for m in range(MT):
                    msl = slice(m * P, (m + 1) * P)
                    ps = ps_pool.tile([P, PANEL], F32, tag="mm")
                    for k in range(KC):
                        lhsT = xT_sb[:, k, msl]
                        for c in range(PANEL // YP):
                            piece = p * (PANEL // YP) + c
                            for j in range(YP // 512):
                                nc.tensor.matmul(
                                    ps[:, c * YP + j * 512:c * YP + (j + 1) * 512],
                                    lhsT,
                                    yT_sb[:, piece, k, j * 512:(j + 1) * 512],
                                    start=(k == 0), stop=(k == KC - 1))
                    st_in = stage_pool.tile([P, PANEL], BF16, tag="stin",
                                            bufs=4)
                    nc.scalar.activation(st_in[:], ps[:], AF.Exp,
                                         bias=negxn[:, m:m + 1], scale=p2g_f[:])
                    st_out = stage_pool.tile([P, PANEL], BF16, tag="stout",
                                             bufs=4)
                    nc.vector.tensor_tensor(st_out[:], st_in[:], Eyb[:, p],
                                            ALU.mult)
                    nc.sync.dma_start(
                        out_d[msl, p * PANEL:(p + 1) * PANEL], st_out[:])
    nc.compile()
    return nc


def _get_nc():
    global _NC
    if _NC is None:
        _NC = _build_nc()
    return _NC


def kernel(x, y, W1, b1, W2, b2):
    global LAST_RESULT
    x = np.asarray(x, dtype=np.float32)
    y = np.asarray(y, dtype=np.float32)
    bf = ml_dtypes.bfloat16

    # y^T piece-major [NP, 128, KC, YP]: [c, p, k, j] = y[c*YP+j, k*128+p]
    yt = np.ascontiguousarray(
        y.T.reshape(KC, P, NP, YP).transpose(2, 1, 0, 3)).astype(bf)

    gp = np.zeros((P, GP_COLS), np.float32)
    w1T = np.asarray(W1, np.float32).T  # (D, HID)
    gp[:, 0:HID] = w1T[0:P]
    gp[:, HID:2 * HID] = w1T[P:2 * P]
    gp[:, 32] = x[0, 0:P]
    gp[:, 33] = x[0, P:2 * P]
    gp[0:HID, 34] = np.asarray(b1, np.float32)
    gp[0:HID, 35] = np.asarray(W2, np.float32).reshape(HID)
    gp[0, 36] = np.asarray(b2, np.float32).reshape(1)[0]

    in_maps = []
    for c in range(N_CORES):
        shard = x[c * N_SH:(c + 1) * N_SH]
        xt = np.ascontiguousarray(
            shard.T.reshape(KC, P, N_SH).transpose(1, 0, 2)).astype(bf)
        xr = np.ascontiguousarray(
            shard.reshape(MT, P, D).transpose(1, 0, 2)).astype(bf)
        in_maps.append({"gp": gp, "xt": xt, "xr": xr, "yt": yt})

    nc = _get_nc()
    LAST_RESULT = run_bass_kernel_spmd(nc, in_maps, core_ids=list(range(N_CORES)))
    out = np.empty((N, M), np.float32)
    for c in range(N_CORES):
        out[c * N_SH:(c + 1) * N_SH] = LAST_RESULT.results[c]["out"]
    return out


# revision 20
# speedup vs baseline: 1.1332x; 1.1332x over previous
"""RBF kernel matrix on 8 TRN2 NeuronCores.

Computes out[i, j] = exp(-gamma * (||x_i||^2 + ||y_j||^2 - 2 x_i.y_j))
with gamma = softplus(MLP(x[0])) + 1e-6, as a Bass/Tile SPMD kernel.

Sharding: rows of x across the 8 cores (1024 rows each); y and the tiny
gamma-net parameters are replicated.  Each core computes its (1024, 8192)
slab of the output; the host concatenates the slabs and widens bf16->f32
(the on-device pipeline is bf16 end-to-end after the exp, so the widening
is exact -- same numbers, half the HBM write traffic).

Per-core device pipeline (keeps the PE free of norm-handling matmuls):
  - gamma chain (TensorE f32 matmuls + ACT relu / exp / ln -> softplus)
    from one packed 19 KB parameter DMA
  - Ey[j] = exp(-gamma*||y_j||^2) built once per panel: DVE squares y^T,
    K=128 matmuls against a constant (-0.5) stationary matrix reduce
    them, ACT exponentiates with the per-partition 2*gamma scale
  - x-row norms from native x via DVE square+accum -> per-partition bias
  - main loop per [128, 2048] tile: psum = x.y (bf16 K=128 matmuls), ACT
    stage = exp(2g*psum - g*||x_i||^2) in bf16, DVE stage *= Ey (bf16 2x
    mode), HWDGE DMA of the bf16 tile to DRAM.
The multiplicative split exp(a+b) = exp(a)*exp(b) is safe here: both
factors are ~exp(-gamma*O(d)) << 1 for this input distribution, so
neither factor can overflow on its own.
"""

import numpy as np
import ml_dtypes

import concourse.bacc as bacc
import concourse.bass as bass  # noqa: F401
import concourse.mybir as mybir
import concourse.tile as tile
from concourse.bass_utils import run_bass_kernel_spmd

N_CORES = 8
N, M, D = 8192, 8192, 256
R_SH, C_SH = 4, 2  # 4x2 core grid: shard x rows AND y columns
N_SH = N // R_SH  # rows of x per core (2048)
M_SH = M // C_SH  # columns of y per core (4096)
HID = 16
P = 128
KC = D // P  # k-chunks (2)
MT = N_SH // P  # m-tiles per core (16)
YP = 1024  # y columns per input-DMA piece
NP = M_SH // YP  # pieces (4)
PANEL = 2048  # main-loop columns per panel / psum tile
NPAN = M_SH // PANEL  # panels (2)
GP_COLS = 37  # packed gamma-net params: w1t|w1t|x0|x0|b1|w2t|b2

F32 = mybir.dt.float32
BF16 = mybir.dt.bfloat16
AF = mybir.ActivationFunctionType
ALU = mybir.AluOpType

_NC = None
LAST_RESULT = None


def _ensure_ntff_hook():
    """Register an ``antenv.axon_hooks`` shim if the image lacks it.

    ``run_bass_kernel_spmd(trace=True)`` under axon imports
    ``antenv.axon_hooks.get_axon_ntff_profile_hook``; some images miss the
    module, which would crash tracing.  Recreate the boot-script hook via
    ctypes against libaxon_pjrt.so, degrading to hook=None when absent.
    """
    import contextlib
    import ctypes
    import os
    import sys
    import types

    try:
        import antenv.axon_hooks  # noqa: F401
        return
    except ImportError:
        pass

    hook = None
    so_path = "/opt/axon/libaxon_pjrt.so"
    if os.path.exists(so_path):
        try:
            lib = ctypes.CDLL(so_path)
            if hasattr(lib, "axon_start_nrt_profile"):
                lib.axon_start_nrt_profile.argtypes = [
                    ctypes.POINTER(ctypes.c_int64), ctypes.c_size_t]
                lib.axon_start_nrt_profile.restype = ctypes.c_int64
                lib.axon_stop_nrt_profile.argtypes = [ctypes.c_char_p]
                lib.axon_stop_nrt_profile.restype = ctypes.c_int64

                @contextlib.contextmanager
                def _hook(output_dir, device_ids):
                    import jax
                    jax.devices()
                    if device_ids:
                        ids = (ctypes.c_int64 * len(device_ids))(*device_ids)
                        rc = lib.axon_start_nrt_profile(ids, len(device_ids))
                    else:
                        rc = lib.axon_start_nrt_profile(None, 0)
                    if rc != 0:
                        raise RuntimeError(f"axon_start_nrt_profile rc={rc}")
                    try:
                        yield
                    finally:
                        n = lib.axon_stop_nrt_profile(str(output_dir).encode())
                        if n <= 0:
                            print(f"ntff profile capture wrote {n} files",
                                  file=sys.stderr)

                hook = _hook
        except OSError:
            hook = None

    mod = types.ModuleType("antenv.axon_hooks")
    mod._hook = hook
    mod.get_axon_ntff_profile_hook = lambda: mod._hook

    def _set(h):
        mod._hook = h

    mod.set_axon_ntff_profile_hook = _set
    sys.modules["antenv.axon_hooks"] = mod
    try:
        import antenv
        antenv.axon_hooks = mod
    except ImportError:
        pass


_ensure_ntff_hook()


def _build_nc():
    nc = bacc.Bacc("TRN2", target_bir_lowering=False, debug=False,
                   num_devices=N_CORES)

    gp_d = nc.dram_tensor("gp", [P, GP_COLS], F32, kind="ExternalInput")
    xt_d = nc.dram_tensor("xt", [P, KC, N_SH], BF16, kind="ExternalInput")
    xr_d = nc.dram_tensor("xr", [P, MT, D], BF16, kind="ExternalInput")
    yt_d = nc.dram_tensor("yt", [NP, P, KC, YP], BF16, kind="ExternalInput")
    out_d = nc.dram_tensor("out", [N_SH, M_SH], BF16, kind="ExternalOutput")

    with tile.TileContext(nc) as tc:
        with (
            tc.tile_pool(name="const", bufs=1) as const,
            tc.tile_pool(name="work", bufs=3) as work,
            tc.tile_pool(name="stage", bufs=3) as stage_pool,
            tc.tile_pool(name="ps", bufs=2, space="PSUM") as ps_pool,
        ):
            # ------------- input DMAs (gamma/y on the Sync HWDGE ring, ----
            # ------------- x tensors on the ACT ring) ---------------------
            gp = const.tile([P, GP_COLS], F32)
            nc.sync.dma_start(gp[:], gp_d[:])
            yT_sb = const.tile([P, NP, KC, YP], BF16)
            for c in range(NP):
                nc.sync.dma_start(yT_sb[:, c], yt_d[c])
            xT_sb = const.tile([P, KC, N_SH], BF16)
            nc.scalar.dma_start(xT_sb[:], xt_d[:])
            xr_sb = const.tile([P, MT, D], BF16)
            nc.scalar.dma_start(xr_sb[:], xr_d[:])

            # DVE constants first: no dependencies, run during the boot gap
            ones_row = const.tile([1, P], F32)
            nc.vector.memset(ones_row[:], 1.0)
            negh = const.tile([P, P], BF16)  # stationary -0.5 for ||y||^2
            nc.vector.memset(negh[:], -0.5)
            wrhs = const.tile([P, 512], BF16)  # junk rhs for PE warm-up
            nc.vector.memset(wrhs[:], 0.0)

            # ---------------- gamma chain ----------------
            ps_h = ps_pool.tile([HID, 1], F32, tag="mm")
            for k in range(KC):
                nc.tensor.matmul(ps_h[:], gp[:, k * HID:(k + 1) * HID],
                                 gp[:, 32 + k:33 + k],
                                 start=(k == 0), stop=(k == KC - 1))
            h_sb = const.tile([HID, 1], F32)  # relu(W1 x0 + b1) on the DVE
            nc.vector.tensor_scalar(h_sb[:], ps_h[:], gp[0:HID, 34:35], 0.0,
                                    ALU.add, ALU.max)

            ps_z = ps_pool.tile([1, 1], F32, tag="mm")
            nc.tensor.matmul(ps_z[:], gp[0:HID, 35:36], h_sb[:],
                             start=True, stop=True)
            u_sb = const.tile([1, 1], F32)
            nc.scalar.activation(u_sb[:], ps_z[:], AF.Exp, bias=gp[0:1, 36:37])
            s_sb = const.tile([1, 1], F32)  # softplus(z) = ln(1 + e^z)
            nc.scalar.activation(s_sb[:], u_sb[:], AF.Ln, bias=1.0)
            # dummy exp: forces the exp table-set reload (after Ln evicted
            # it) to happen here, off the Ey critical path
            dummy_e = const.tile([1, 1], F32)
            nc.scalar.activation(dummy_e[:], u_sb[:], AF.Exp)

            ps_g = ps_pool.tile([P, 1], F32, tag="mm")
            nc.tensor.matmul(ps_g[:], ones_row[:], s_sb[:], start=True, stop=True)

            # PE warm-up: ~3.4us of junk matmuls during the otherwise-idle
            # input-load window flips the HAM clock gate to 8/8 before the
            # first real K=128 matmuls (negh is the only dep; no readers)
            warm_ps = ps_pool.tile([P, 512], F32, tag="mm")
            for w in range(8):
                nc.tensor.matmul(warm_ps[:], negh[:], wrhs[:],
                                 start=True, stop=True, skip_group_check=True)

            negg_f = const.tile([P, 1], F32)     # -gamma on every partition
            nc.vector.tensor_scalar(negg_f[:], ps_g[:], -1.0, -1e-6,
                                    ALU.mult, ALU.add)
            p2g_f = const.tile([P, 1], F32)      # +2*gamma
            nc.vector.tensor_scalar(p2g_f[:], ps_g[:], 2.0, 2e-6,
                                    ALU.mult, ALU.add)

            xn = const.tile([P, MT], F32)
            negxn = const.tile([P, MT], F32)     # -gamma * ||x_i||^2
            Eyb = const.tile([P, NPAN, PANEL], BF16)

            # ---------------- panels: Ey prep + main loop -----------------
            for p in range(NPAN):
                # sqy + (-0.5)-matmuls for Ey of this panel's columns; these
                # need neither gamma nor x, so they fill the prolog
                ps_b = ps_pool.tile([P, PANEL], F32, tag="mm")
                for c in range(PANEL // YP):
                    piece = p * (PANEL // YP) + c
                    sqy = work.tile([P, KC, YP], BF16, tag="sqy")
                    nc.vector.tensor_tensor(sqy[:], yT_sb[:, piece],
                                            yT_sb[:, piece], ALU.mult)
                    for k in range(KC):
                        for j in range(YP // 512):
                            nc.tensor.matmul(
                                ps_b[:, c * YP + j * 512:c * YP + (j + 1) * 512],
                                negh[:], sqy[:, k, j * 512:(j + 1) * 512],
                                start=(k == 0), stop=(k == KC - 1))

                # Ey[j] = exp(-gamma*||y_j||^2) for this panel's columns
                nc.scalar.activation(Eyb[:, p], ps_b[:], AF.Exp,
                                     scale=p2g_f[:])

                if p == 0:
                    # x-row norms -> per-partition bias (needed by the first
                    # main-loop exp; emitted after panel-0 sqy so the DVE
                    # queue never stalls on the slower xr DMA)
                    for m in range(MT):
                        sq_scr = work.tile([P, D], F32, tag="sqx")
                        nc.vector.scalar_tensor_tensor(
                            sq_scr[:], xr_sb[:, m], 1.0, xr_sb[:, m],
                            ALU.mult, ALU.mult, accum_out=xn[:, m:m + 1])
                    nc.vector.tensor_scalar(negxn[:], xn[:], negg_f[:], None,
                                            ALU.mult)

                for m in range(MT):
                    msl = slice(m * P, (m + 1) * P)
                    ps = ps_pool.tile([P, PANEL], F32, tag="mm")
                    for k in range(KC):
                        lhsT = xT_sb[:, k, msl]
                        for c in range(PANEL // YP):
                            piece = p * (PANEL // YP) + c
                            for j in range(YP // 512):
                                nc.tensor.matmul(
                                    ps[:, c * YP + j * 512:c * YP + (j + 1) * 512],
                                    lhsT,
                                    yT_sb[:, piece, k, j * 512:(j + 1) * 512],
                                    start=(k == 0), stop=(k == KC - 1))
                    st_in = stage_pool.tile([P, PANEL], BF16, tag="stin",
                                            bufs=4)
                    nc.scalar.activation(st_in[:], ps[:], AF.Exp,
                                         bias=negxn[:, m:m + 1], scale=p2g_f[:])
                    st_out = stage_pool.tile([P, PANEL], BF16, tag="stout",
                                             bufs=4)
                    nc.vector.tensor_tensor(st_out[:], st_in[:], Eyb[:, p],
                                            ALU.mult)
                    nc.sync.dma_start(
                        out_d[msl, p * PANEL:(p + 1) * PANEL], st_out[:])
    nc.compile()
    return nc


def _get_nc():
    global _NC
    if _NC is None:
        _NC = _build_nc()
    return _NC


def kernel(x, y, W1, b1, W2, b2):
    global LAST_RESULT
    x = np.asarray(x, dtype=np.float32)
    y = np.asarray(y, dtype=np.float32)
    bf = ml_dtypes.bfloat16

    # y^T piece-major [NP, 128, KC, YP] per column shard:
    # [c, p, k, j] = ysub[c*YP+j, k*128+p]
    yts = []
    for q in range(C_SH):
        ysub = y[q * M_SH:(q + 1) * M_SH]
        yts.append(np.ascontiguousarray(
            ysub.T.reshape(KC, P, NP, YP).transpose(2, 1, 0, 3)).astype(bf))

    gp = np.zeros((P, GP_COLS), np.float32)
    w1T = np.asarray(W1, np.float32).T  # (D, HID)
    gp[:, 0:HID] = w1T[0:P]
    gp[:, HID:2 * HID] = w1T[P:2 * P]
    gp[:, 32] = x[0, 0:P]
    gp[:, 33] = x[0, P:2 * P]
    gp[0:HID, 34] = np.asarray(b1, np.float32)
    gp[0:HID, 35] = np.asarray(W2, np.float32).reshape(HID)
    gp[0, 36] = np.asarray(b2, np.float32).reshape(1)[0]

    xts, xrs = [], []
    for r in range(R_SH):
        shard = x[r * N_SH:(r + 1) * N_SH]
        xts.append(np.ascontiguousarray(
            shard.T.reshape(KC, P, N_SH).transpose(1, 0, 2)).astype(bf))
        xrs.append(np.ascontiguousarray(
            shard.reshape(MT, P, D).transpose(1, 0, 2)).astype(bf))

    in_maps = []
    for c in range(N_CORES):
        r, q = divmod(c, C_SH)
        in_maps.append({"gp": gp, "xt": xts[r], "xr": xrs[r], "yt": yts[q]})

    nc = _get_nc()
    LAST_RESULT = run_bass_kernel_spmd(nc, in_maps, core_ids=list(range(N_CORES)))
    out = np.empty((N, M), np.float32)
    for c in range(N_CORES):
        r, q = divmod(c, C_SH)
        out[r * N_SH:(r + 1) * N_SH,
            q * M_SH:(q + 1) * M_SH] = LAST_RESULT.results[c]["out"]
    return out


# revision 23
# speedup vs baseline: 1.1674x; 1.0302x over previous
"""RBF kernel matrix on 8 TRN2 NeuronCores.

Computes out[i, j] = exp(-gamma * (||x_i||^2 + ||y_j||^2 - 2 x_i.y_j))
with gamma = softplus(MLP(x[0])) + 1e-6, as a Bass/Tile SPMD kernel.

Sharding: 4x2 core grid -- x rows in 4 shards of 2048, y columns in 2
shards of 4096 (minimizes per-core input bytes vs pure row sharding and
halves the per-core Ey prep); the tiny gamma-net parameters are
replicated and every core recomputes gamma from x[0].  Each core
computes its (2048, 4096) block of the output; the host assembles the
blocks and widens bf16->f32 (the on-device pipeline is bf16 end-to-end
after the exp, so the widening is exact -- same numbers, half the HBM
write traffic).

Per-core device pipeline (keeps the PE free of norm-handling matmuls):
  - gamma chain (TensorE f32 matmuls + ACT relu / exp / ln -> softplus)
    from one packed 19 KB parameter DMA
  - Ey[j] = exp(-gamma*||y_j||^2) built once per panel: DVE squares y^T,
    K=128 matmuls against a constant (-0.5) stationary matrix reduce
    them, ACT exponentiates with the per-partition 2*gamma scale
  - x-row norms from native x via DVE square+accum -> per-partition bias
  - main loop per [128, 2048] tile: psum = x.y (bf16 K=128 matmuls), ACT
    stage = exp(2g*psum - g*||x_i||^2) in bf16, DVE stage *= Ey (bf16 2x
    mode), HWDGE DMA of the bf16 tile to DRAM.
The multiplicative split exp(a+b) = exp(a)*exp(b) is safe here: both
factors are ~exp(-gamma*O(d)) << 1 for this input distribution, so
neither factor can overflow on its own.
"""

import numpy as np
import ml_dtypes

import concourse.bacc as bacc
import concourse.bass as bass  # noqa: F401
import concourse.mybir as mybir
import concourse.tile as tile
from concourse.bass_utils import run_bass_kernel_spmd

N_CORES = 8
N, M, D = 8192, 8192, 256
R_SH, C_SH = 4, 2  # 4x2 core grid: shard x rows AND y columns
N_SH = N // R_SH  # rows of x per core (2048)
M_SH = M // C_SH  # columns of y per core (4096)
HID = 16
P = 128
KC = D // P  # k-chunks (2)
MT = N_SH // P  # m-tiles per core (16)
YP = 1024  # y columns per input-DMA piece
NP = M_SH // YP  # pieces (4)
PANEL = 2048  # main-loop columns per panel / psum tile
NPAN = M_SH // PANEL  # panels (2)
GP_COLS = 37  # packed gamma-net params: w1t|w1t|x0|x0|b1|w2t|b2

F32 = mybir.dt.float32
BF16 = mybir.dt.bfloat16
AF = mybir.ActivationFunctionType
ALU = mybir.AluOpType

_NC = None
LAST_RESULT = None


def _ensure_ntff_hook():
    """Register an ``antenv.axon_hooks`` shim if the image lacks it.

    ``run_bass_kernel_spmd(trace=True)`` under axon imports
    ``antenv.axon_hooks.get_axon_ntff_profile_hook``; some images miss the
    module, which would crash tracing.  Recreate the boot-script hook via
    ctypes against libaxon_pjrt.so, degrading to hook=None when absent.
    """
    import contextlib
    import ctypes
    import os
    import sys
    import types

    try:
        import antenv.axon_hooks  # noqa: F401
        return
    except ImportError:
        pass

    hook = None
    so_path = "/opt/axon/libaxon_pjrt.so"
    if os.path.exists(so_path):
        try:
            lib = ctypes.CDLL(so_path)
            if hasattr(lib, "axon_start_nrt_profile"):
                lib.axon_start_nrt_profile.argtypes = [
                    ctypes.POINTER(ctypes.c_int64), ctypes.c_size_t]
                lib.axon_start_nrt_profile.restype = ctypes.c_int64
                lib.axon_stop_nrt_profile.argtypes = [ctypes.c_char_p]
                lib.axon_stop_nrt_profile.restype = ctypes.c_int64

                @contextlib.contextmanager
                def _hook(output_dir, device_ids):
                    import jax
                    jax.devices()
                    if device_ids:
                        ids = (ctypes.c_int64 * len(device_ids))(*device_ids)
                        rc = lib.axon_start_nrt_profile(ids, len(device_ids))
                    else:
                        rc = lib.axon_start_nrt_profile(None, 0)
                    if rc != 0:
                        raise RuntimeError(f"axon_start_nrt_profile rc={rc}")
                    try:
                        yield
                    finally:
                        n = lib.axon_stop_nrt_profile(str(output_dir).encode())
                        if n <= 0:
                            print(f"ntff profile capture wrote {n} files",
                                  file=sys.stderr)

                hook = _hook
        except OSError:
            hook = None

    mod = types.ModuleType("antenv.axon_hooks")
    mod._hook = hook
    mod.get_axon_ntff_profile_hook = lambda: mod._hook

    def _set(h):
        mod._hook = h

    mod.set_axon_ntff_profile_hook = _set
    sys.modules["antenv.axon_hooks"] = mod
    try:
        import antenv
        antenv.axon_hooks = mod
    except ImportError:
        pass


_ensure_ntff_hook()


def _build_nc():
    nc = bacc.Bacc("TRN2", target_bir_lowering=False, debug=False,
                   num_devices=N_CORES)

    gp_d = nc.dram_tensor("gp", [P, GP_COLS], F32, kind="ExternalInput")
    xt_d = nc.dram_tensor("xt", [P, KC, N_SH], BF16, kind="ExternalInput")
    xr_d = nc.dram_tensor("xr", [P, MT, D], BF16, kind="ExternalInput")
    yt_d = nc.dram_tensor("yt", [NP, P, KC, YP], BF16, kind="ExternalInput")
    out_d = nc.dram_tensor("out", [N_SH, M_SH], BF16, kind="ExternalOutput")

    with tile.TileContext(nc) as tc:
        with (
            tc.tile_pool(name="const", bufs=1) as const,
            tc.tile_pool(name="work", bufs=3) as work,
            tc.tile_pool(name="stage", bufs=3) as stage_pool,
            tc.tile_pool(name="ps", bufs=2, space="PSUM") as ps_pool,
        ):
            # ------------- input DMAs (gamma/y on the Sync HWDGE ring, ----
            # ------------- x tensors on the ACT ring) ---------------------
            gp = const.tile([P, GP_COLS], F32)
            nc.sync.dma_start(gp[:], gp_d[:])
            yT_sb = const.tile([P, NP, KC, YP], BF16)
            for c in range(NP):
                nc.sync.dma_start(yT_sb[:, c], yt_d[c])
            xT_sb = const.tile([P, KC, N_SH], BF16)
            nc.scalar.dma_start(xT_sb[:], xt_d[:])
            xr_sb = const.tile([P, MT, D], BF16)
            nc.scalar.dma_start(xr_sb[:], xr_d[:])

            # DVE constants first: no dependencies, run during the boot gap
            ones_row = const.tile([1, P], F32)
            nc.vector.memset(ones_row[:], 1.0)
            negh = const.tile([P, P], BF16)  # stationary -0.5 for ||y||^2
            nc.vector.memset(negh[:], -0.5)
            wrhs = const.tile([P, 512], BF16)  # junk rhs for PE warm-up
            nc.vector.memset(wrhs[:], 0.0)

            # ---------------- gamma chain ----------------
            ps_h = ps_pool.tile([HID, 1], F32, tag="mm")
            for k in range(KC):
                nc.tensor.matmul(ps_h[:], gp[:, k * HID:(k + 1) * HID],
                                 gp[:, 32 + k:33 + k],
                                 start=(k == 0), stop=(k == KC - 1))
            h_sb = const.tile([HID, 1], F32)  # relu(W1 x0 + b1) on the DVE
            nc.vector.tensor_scalar(h_sb[:], ps_h[:], gp[0:HID, 34:35], 0.0,
                                    ALU.add, ALU.max)

            ps_z = ps_pool.tile([1, 1], F32, tag="mm")
            nc.tensor.matmul(ps_z[:], gp[0:HID, 35:36], h_sb[:],
                             start=True, stop=True)
            u_sb = const.tile([1, 1], F32)
            nc.scalar.activation(u_sb[:], ps_z[:], AF.Exp, bias=gp[0:1, 36:37])
            s_sb = const.tile([1, 1], F32)  # softplus(z) = ln(1 + e^z)
            nc.scalar.activation(s_sb[:], u_sb[:], AF.Ln, bias=1.0)
            # dummy exp: forces the exp table-set reload (after Ln evicted
            # it) to happen here, off the Ey critical path
            dummy_e = const.tile([1, 1], F32)
            nc.scalar.activation(dummy_e[:], u_sb[:], AF.Exp)

            ps_g = ps_pool.tile([P, 1], F32, tag="mm")
            nc.tensor.matmul(ps_g[:], ones_row[:], s_sb[:], start=True, stop=True)

            # PE warm-up: ~3.4us of junk matmuls gated on the first y piece,
            # so they run right before the first real K=128 matmuls and flip
            # the HAM clock gate to 8/8 for them (results are never read)
            warm_ps = ps_pool.tile([P, 512], F32, tag="mm")
            for w in range(8):
                nc.tensor.matmul(warm_ps[:], negh[:], yT_sb[:, 0, 0, 0:512],
                                 start=True, stop=True, skip_group_check=True)

            negg_f = const.tile([P, 1], F32)     # -gamma on every partition
            nc.vector.tensor_scalar(negg_f[:], ps_g[:], -1.0, -1e-6,
                                    ALU.mult, ALU.add)
            p2g_f = const.tile([P, 1], F32)      # +2*gamma
            nc.vector.tensor_scalar(p2g_f[:], ps_g[:], 2.0, 2e-6,
                                    ALU.mult, ALU.add)

            xn = const.tile([P, MT], F32)
            negxn = const.tile([P, MT], F32)     # -gamma * ||x_i||^2
            Eyb = const.tile([P, NPAN, PANEL], BF16)

            # ---------------- panels: Ey prep + main loop -----------------
            for p in range(NPAN):
                # sqy + (-0.5)-matmuls for Ey of this panel's columns; these
                # need neither gamma nor x, so they fill the prolog
                ps_b = ps_pool.tile([P, PANEL], F32, tag="mm")
                for c in range(PANEL // YP):
                    piece = p * (PANEL // YP) + c
                    sqy = work.tile([P, KC, YP], BF16, tag="sqy")
                    for k in range(KC):
                        # per-k sqy ops so the first matmul pair can start
                        # one DVE-drain earlier
                        nc.vector.tensor_tensor(sqy[:, k], yT_sb[:, piece, k],
                                                yT_sb[:, piece, k], ALU.mult)
                        for j in range(YP // 512):
                            nc.tensor.matmul(
                                ps_b[:, c * YP + j * 512:c * YP + (j + 1) * 512],
                                negh[:], sqy[:, k, j * 512:(j + 1) * 512],
                                start=(k == 0), stop=(k == KC - 1))

                # Ey[j] = exp(-gamma*||y_j||^2) for this panel's columns
                nc.scalar.activation(Eyb[:, p], ps_b[:], AF.Exp,
                                     scale=p2g_f[:])

                if p == 0:
                    # x-row norms -> per-partition bias (needed by the first
                    # main-loop exp; emitted after panel-0 sqy so the DVE
                    # queue never stalls on the slower xr DMA)
                    for m in range(MT):
                        sq_scr = work.tile([P, D], F32, tag="sqx")
                        nc.vector.scalar_tensor_tensor(
                            sq_scr[:], xr_sb[:, m], 1.0, xr_sb[:, m],
                            ALU.mult, ALU.mult, accum_out=xn[:, m:m + 1])
                    nc.vector.tensor_scalar(negxn[:], xn[:], negg_f[:], None,
                                            ALU.mult)

                for m in range(MT):
                    msl = slice(m * P, (m + 1) * P)
                    ps = ps_pool.tile([P, PANEL], F32, tag="mm")
                    for k in range(KC):
                        lhsT = xT_sb[:, k, msl]
                        for c in range(PANEL // YP):
                            piece = p * (PANEL // YP) + c
                            for j in range(YP // 512):
                                nc.tensor.matmul(
                                    ps[:, c * YP + j * 512:c * YP + (j + 1) * 512],
                                    lhsT,
                                    yT_sb[:, piece, k, j * 512:(j + 1) * 512],
                                    start=(k == 0), stop=(k == KC - 1))
                    st_in = stage_pool.tile([P, PANEL], BF16, tag="stin",
                                            bufs=4)
                    nc.scalar.activation(st_in[:], ps[:], AF.Exp,
                                         bias=negxn[:, m:m + 1], scale=p2g_f[:])
                    st_out = stage_pool.tile([P, PANEL], BF16, tag="stout",
                                             bufs=4)
                    nc.vector.tensor_tensor(st_out[:], st_in[:], Eyb[:, p],
                                            ALU.mult)
                    nc.sync.dma_start(
                        out_d[msl, p * PANEL:(p + 1) * PANEL], st_out[:])
    nc.compile()
    return nc


def _get_nc():
    global _NC
    if _NC is None:
        _NC = _build_nc()
    return _NC


def kernel(x, y, W1, b1, W2, b2):
    global LAST_RESULT
    x = np.asarray(x, dtype=np.float32)
    y = np.asarray(y, dtype=np.float32)
    bf = ml_dtypes.bfloat16

    # y^T piece-major [NP, 128, KC, YP] per column shard:
    # [c, p, k, j] = ysub[c*YP+j, k*128+p]
    yts = []
    for q in range(C_SH):
        ysub = y[q * M_SH:(q + 1) * M_SH]
        yts.append(np.ascontiguousarray(
            ysub.T.reshape(KC, P, NP, YP).transpose(2, 1, 0, 3)).astype(bf))

    gp = np.zeros((P, GP_COLS), np.float32)
    w1T = np.asarray(W1, np.float32).T  # (D, HID)
    gp[:, 0:HID] = w1T[0:P]
    gp[:, HID:2 * HID] = w1T[P:2 * P]
    gp[:, 32] = x[0, 0:P]
    gp[:, 33] = x[0, P:2 * P]
    gp[0:HID, 34] = np.asarray(b1, np.float32)
    gp[0:HID, 35] = np.asarray(W2, np.float32).reshape(HID)
    gp[0, 36] = np.asarray(b2, np.float32).reshape(1)[0]

    xts, xrs = [], []
    for r in range(R_SH):
        shard = x[r * N_SH:(r + 1) * N_SH]
        xts.append(np.ascontiguousarray(
            shard.T.reshape(KC, P, N_SH).transpose(1, 0, 2)).astype(bf))
        xrs.append(np.ascontiguousarray(
            shard.reshape(MT, P, D).transpose(1, 0, 2)).astype(bf))

    in_maps = []
    for c in range(N_CORES):
        r, q = divmod(c, C_SH)
        in_maps.append({"gp": gp, "xt": xts[r], "xr": xrs[r], "yt": yts[q]})

    nc = _get_nc()
    LAST_RESULT = run_bass_kernel_spmd(nc, in_maps, core_ids=list(range(N_CORES)))
    out = np.empty((N, M), np.float32)
    for c in range(N_CORES):
        r, q = divmod(c, C_SH)
        out[r * N_SH:(r + 1) * N_SH,
            q * M_SH:(q + 1) * M_SH] = LAST_RESULT.results[c]["out"]
    return out
